# revision 53
# baseline (speedup 1.0000x reference)
"""Gated multi-head attention (AlphaFold-style) on 8 TRN2 NeuronCores.

Sharding: data-parallel over batch B=32 -> 4 batches per core; zero collectives.

v2 over the v1 baseline (145us):
  - All DMA layouts packed host-side so every per-partition line is
    contiguous (2KB-32KB): one DMA per (batch, kc) for batched bias
    (8KB lines), one DMA for the whole nonbatched bias (32KB lines).
  - Bias pre-add (nbb = nbT + bbT) split by head quad: heads 0-3 on DVE
    (bf16 2x mode), heads 4-7 on GPSIMD (otherwise idle engine).
  - Softmax denominators: the ones-matmuls now target the column bands
    the AV matmuls are NOT using (band j^2), so all 4 matmuls of an
    emit_av group run concurrently in the PE array. Sums land with
    32-row bands swapped (rows 64-127 hold heads' j=0,1 sums); the
    post chain compensates with 64-partition-offset DVE operands.
  - PSUM->SBUF evacuation copies (qhT/khT/vb) moved to ACT (idle during
    the projection phase); sigmoid/copies merged into wider single ops.

  qhT[hc, q]  = (query_w*scale)[a,hc]^T @ q_dataT[a,q]           (PE)
  khT[hc, k]  =  key_w^T @ m_dataT                               (PE)
  vb[k, hc]   =  (m_dataT^T-chunks @ value_w) -> bf16            (PE + ACT)
  gateT[hc,q] =  sigmoid(gating_w^T @ q_dataT + gating_b)        (PE + ACT)
  logitsT(h)[k,q] = khT_h^T-slices @ qhT_h  (row-tiled)          (PE)
  psum += Id @ (nbT + bbT)               (PE id-add; DVE/GPS pre-add)
  PT(h)[k,q]  = exp(psum + bias_row[k])  -> bf16                 (ACT)
  avT, sums   = col-tiled matmuls over k, all 4 bands concurrent (PE)
  wag         = avT * gateT * approx(1/sums)                     (DVE)
  outT[o, q]  = output_w^T-chunks @ wag + output_b               (PE + DVE)
"""

import numpy as np

import concourse.bass as bass
import concourse.mybir as mybir
from concourse import bacc
from concourse.tile import TileContext
from concourse.masks import make_identity
from concourse.bass_utils import run_bass_kernel_spmd

B, Q, K, A, H, C, O = 32, 512, 512, 256, 8, 32, 256
CORES = 8
BLOC = B // CORES          # batches per core
NKC = K // 128             # k chunks
F32 = mybir.dt.float32
BF16 = mybir.dt.bfloat16
KEY_SCALE = float(C) ** -0.5
AF = mybir.ActivationFunctionType
ALU = mybir.AluOpType


def build_nc():
    nc = bacc.Bacc(None, target_bir_lowering=False)

    # --- DRAM parameters (per-core shards; packed layouts, see make_in_maps) ---
    p_qT = nc.declare_dram_parameter("qT", [BLOC, 128, 2, Q], BF16, isOutput=False)
    p_mT = nc.declare_dram_parameter("mT", [BLOC, 128, 2, K], BF16, isOutput=False)
    p_br = nc.declare_dram_parameter("biasr", [BLOC, 128, NKC], F32, isOutput=False)
    p_bbT = nc.declare_dram_parameter("bbT", [BLOC, NKC, 128, H, Q], BF16,
                                      isOutput=False)
    p_nbT = nc.declare_dram_parameter("nbT", [128, NKC, H, Q], BF16, isOutput=False)
    p_qw = nc.declare_dram_parameter("qw", [128, 2, 256], BF16, isOutput=False)
    p_kw = nc.declare_dram_parameter("kw", [128, 2, 256], BF16, isOutput=False)
    p_vw = nc.declare_dram_parameter("vw", [128, 2, 256], BF16, isOutput=False)
    p_gw = nc.declare_dram_parameter("gw", [128, 2, 256], BF16, isOutput=False)
    p_gb = nc.declare_dram_parameter("gb", [128, 2], F32, isOutput=False)
    p_ow = nc.declare_dram_parameter("ow", [128, 2, 256], BF16, isOutput=False)
    p_ob = nc.declare_dram_parameter("ob", [128, 2], F32, isOutput=False)
    p_out = nc.declare_dram_parameter("out", [BLOC, 128, 2, Q], BF16, isOutput=True)

    with TileContext(nc) as tc:
        with (
            tc.tile_pool(name="const", bufs=1) as const,
            tc.tile_pool(name="nbres", bufs=1) as nbres,
            tc.tile_pool(name="data", bufs=4) as data,
            tc.tile_pool(name="proj", bufs=4) as proj,
            tc.tile_pool(name="bbt", bufs=4) as bbtp,
            tc.tile_pool(name="nbb", bufs=4) as nbbp,
            tc.tile_pool(name="pt", bufs=8) as ptp,
            tc.tile_pool(name="post", bufs=1) as post,
            tc.tile_pool(name="ps", bufs=3, space="PSUM") as psp,
            tc.tile_pool(name="avps", bufs=1, space="PSUM") as avps,
            tc.tile_pool(name="sumps", bufs=1, space="PSUM") as sumps,
        ):
            # ---------- one-time constants ----------
            nbt16 = nbres.tile([128, NKC, H, Q], BF16)

            ident = const.tile([128, 128], BF16)
            make_identity(nc, ident)
            ones = const.tile([128, 32], BF16)
            nc.vector.memset(ones, 1.0)

            # weights: packed [128, ka, hc]; cast to bf16 on device;
            # key_scale folded into qw here.
            qw_sb = const.tile([128, 2, 256], BF16)
            kw_sb = const.tile([128, 2, 256], BF16)
            vw_sb = const.tile([128, 2, 256], BF16)
            gw_sb = const.tile([128, 2, 256], BF16)
            ow_sb = const.tile([128, 2, 256], BF16)
            for t, p in ((kw_sb, p_kw), (vw_sb, p_vw),
                         (gw_sb, p_gw), (ow_sb, p_ow)):
                nc.sync.dma_start(out=t, in_=p[:])
            wstage = data.tile([128, 2, 256], BF16, tag="stage", bufs=1)
            nc.sync.dma_start(out=wstage, in_=p_qw[:])
            nc.vector.tensor_scalar_mul(out=qw_sb, in0=wstage, scalar1=KEY_SCALE)
            gb_sb = const.tile([128, 2], F32)
            nc.sync.dma_start(out=gb_sb, in_=p_gb[:])
            ob_sb = const.tile([128, 2], F32)
            nc.sync.dma_start(out=ob_sb, in_=p_ob[:])
            # negated gating bias: the gate is computed through the Exp
            # table (sigmoid(x) = 1/(1+exp(-x))) so the attention phase
            # never needs an ACT table switch
            ngb_sb = const.tile([128, 2], F32)
            nc.vector.tensor_scalar_mul(out=ngb_sb, in0=gb_sb, scalar1=-1.0)

            # ---------- per-batch pipeline ----------
            # The post chain is split into pieces emitted across the first
            # quads of the NEXT batch so its DVE work never forms a solid
            # block in front of that batch's bias pre-adds.
            def make_post(b, avt, smt, gate):
                recb = [None]
                wag = [None]

                def piece1():
                    recb[0] = post.tile([128, 2, Q], F32, tag="recb", name="recb")
                    grec = post.tile([128, 2, Q], F32, tag="grec")
                    for t in range(2):
                        nc.vector.reciprocal_approx_fast(
                            out=recb[0][:, t], in_=smt[t])
                    for t in range(2):
                        nc.vector.tensor_mul(
                            out=grec[:, t], in0=gate[:, t], in1=recb[0][:, t])
                    recb[0] = grec

                def piece2():
                    grec = recb[0]
                    wag[0] = post.tile([128, 2, Q], BF16, tag="wag", name="wag")
                    for t in range(2):
                        nc.vector.tensor_mul(
                            out=wag[0][:, t], in0=avt[t], in1=grec[:, t])

                def piece3():
                    outT = post.tile([128, 2, Q], BF16, tag="outT", bufs=2)
                    po2 = psp.tile([128, 2, Q], F32, tag="mm")
                    for mo in range(2):
                        oslc = slice(mo * 128, (mo + 1) * 128)
                        for kh in range(2):
                            nc.tensor.matmul(
                                po2[:, mo], ow_sb[:, kh, oslc], wag[0][:, kh],
                                start=(kh == 0), stop=(kh == 1))
                    for mo in range(2):
                        nc.scalar.add(out=outT[:, mo], in_=po2[:, mo],
                                      add=ob_sb[:, mo:mo + 1])
                    nc.gpsimd.dma_start(out=p_out[b], in_=outT)

                return [piece1, piece2, piece3]

            # ---------- hoisted input DMA loads ----------
            br_l, qT_l, mT_l = [], [], []
            for b in range(BLOC):
                qT_sb = data.tile([128, 2, Q], BF16, tag="qT", bufs=4)
                nc.sync.dma_start(out=qT_sb, in_=p_qT[b])
                mT_sb = data.tile([128, 2, K], BF16, tag="mT", bufs=2)
                nc.sync.dma_start(out=mT_sb, in_=p_mT[b])
                br_sb = data.tile([128, NKC], F32, tag="br")
                nc.sync.dma_start(out=br_sb, in_=p_br[b])
                qT_l.append(qT_sb)
                mT_l.append(mT_sb)
                br_l.append(br_sb)
                if b == 0:
                    # batch 0's attention starts right after proj(0); its
                    # first bias pre-adds need the kc0 chunk early
                    nc.sync.dma_start(out=nbt16[:, 0, 0:4], in_=p_nbT[:, 0, 0:4])
                    nc.sync.dma_start(out=nbt16[:, 0, 4:8], in_=p_nbT[:, 0, 4:8])
            for kc in range(1, NKC):
                nc.sync.dma_start(out=nbt16[:, kc, 0:4], in_=p_nbT[:, kc, 0:4])
                nc.sync.dma_start(out=nbt16[:, kc, 4:8], in_=p_nbT[:, kc, 4:8])

            # ---------- per-batch projections ----------
            # Emitted at the tail of the previous batch's attention: the PE
            # fills the batch-boundary bubble with proj matmuls (HAM stays
            # warm), ACT takes the PSUM evacuations there (it idles at the
            # boundary), and the gate goes through Exp so no table switch.
            def make_proj(b, use_sigmoid=True):
                qT_sb, mT_sb = qT_l[b], mT_l[b]
                qhT = proj.tile([128, 2, Q], BF16, tag="qhT", bufs=4, name="qhT")
                khT = proj.tile([128, 2, K], BF16, tag="khT", bufs=4, name="khT")
                gate = proj.tile([128, 2, Q], F32, tag="gate", bufs=4, name="gate")
                pqh = psp.tile([128, 2, Q], F32, tag="mm", name="pqh")
                pkh = psp.tile([128, 2, Q], F32, tag="mm", name="pkh")
                pgh = psp.tile([128, 2, Q], F32, tag="mm", name="pgh")
                for m in range(2):
                    mslc = slice(m * 128, (m + 1) * 128)
                    for ka in range(2):
                        st, sp = ka == 0, ka == 1
                        nc.tensor.matmul(
                            pqh[:, m], qw_sb[:, ka, mslc], qT_sb[:, ka],
                            start=st, stop=sp)
                        nc.tensor.matmul(
                            pkh[:, m], kw_sb[:, ka, mslc], mT_sb[:, ka],
                            start=st, stop=sp)
                        nc.tensor.matmul(
                            pgh[:, m], gw_sb[:, ka, mslc], qT_sb[:, ka],
                            start=st, stop=sp)
                nc.vector.tensor_copy(out=qhT, in_=pqh)
                nc.vector.tensor_copy(out=khT, in_=pkh)
                if use_sigmoid:
                    for m in range(2):
                        nc.scalar.activation(gate[:, m], pgh[:, m], AF.Sigmoid,
                                             bias=gb_sb[:, m:m + 1], scale=1.0)
                else:
                    # gate via the resident Exp table: 1/(1+exp(-(x+gb)))
                    for m in range(2):
                        nc.scalar.activation(gate[:, m], pgh[:, m], AF.Exp,
                                             bias=ngb_sb[:, m:m + 1], scale=-1.0)
                    nc.vector.tensor_scalar_add(out=gate, in0=gate, scalar1=1.0)
                    nc.vector.reciprocal_approx_fast(out=gate, in_=gate)

                vb = proj.tile([128, NKC, 256], BF16, tag="vb", bufs=4, name="vb")
                for kch in range(2):
                    pv2 = psp.tile([128, 2, Q], F32, tag="mm", name="pv2")
                    for kci in range(2):
                        kc = 2 * kch + kci
                        kslc = slice(kc * 128, (kc + 1) * 128)
                        pv = pv2[:, kci, 0:256]
                        for ka in range(2):
                            nc.tensor.matmul(
                                pv, mT_sb[:, ka, kslc], vw_sb[:, ka],
                                start=(ka == 0), stop=(ka == 1))
                    nc.vector.tensor_copy(out=vb[:, 2 * kch:2 * kch + 2],
                                       in_=pv2[:, :, 0:256])
                return qhT, khT, gate, vb

            nbb_ready = {}

            def prep_bias(b2, kc2):
                bbt = bbtp.tile([128, H, Q], BF16, tag="bbt", name="bbt")
                nc.sync.dma_start(out=bbt, in_=p_bbT[b2, kc2])
                nbb = nbbp.tile([128, H, Q], BF16, tag="nbb", name="nbb")
                nc.vector.tensor_add(
                    out=nbb[:, 0:4], in0=nbt16[:, kc2, 0:4], in1=bbt[:, 0:4])
                nc.vector.tensor_add(
                    out=nbb[:, 4:8], in0=nbt16[:, kc2, 4:8], in1=bbt[:, 4:8])
                nbb_ready[(b2, kc2)] = nbb

            projd_l = [make_proj(0), make_proj(1), None, None]
            prep_bias(0, 0)
            pending_post = None
            for b in range(BLOC):
                if b == 1:
                    # emitted here, these run inside attn(b0)'s PE slack and
                    # their sigmoids land in the b0->b1 boundary bubble
                    projd_l[2] = make_proj(2)
                    projd_l[3] = make_proj(3)
                qhT, khT, gate, vb = projd_l[b]
                br_sb = br_l[b]

                # --- attention core ---
                av0 = avps.tile([128, Q], F32, tag="av")     # heads 0-3
                av1 = avps.tile([128, Q], F32, tag="av")     # heads 4-7
                sm0 = sumps.tile([128, Q], F32, tag="sm")    # band-swapped
                sm1 = sumps.tile([128, Q], F32, tag="sm")
                avt = (av0, av1)
                smt = (sm0, sm1)

                def emit_av(g):
                    # 4 heads in 4 distinct column bands -> concurrent.
                    g_heads, g_pts, g_kc = g
                    for i2, h2 in enumerate(g_heads):
                        j2 = h2 % 4
                        nc.tensor.matmul(
                            avt[h2 // 4][32 * j2:32 * j2 + 32],
                            vb[:, g_kc, 32 * h2:32 * h2 + 32],
                            g_pts[i2],
                            start=(g_kc == 0), stop=(g_kc == NKC - 1),
                            tile_position=(0, 32 * j2), skip_group_check=True)

                def emit_sums(g):
                    g_heads, g_pts, g_kc = g
                    for i2, h2 in enumerate(g_heads):
                        j2 = h2 % 4
                        nc.tensor.matmul(
                            smt[h2 // 4][32 * j2:32 * j2 + 32],
                            ones, g_pts[i2],
                            start=(g_kc == 0), stop=(g_kc == NKC - 1),
                            tile_position=(0, 32 * j2), skip_group_check=True)

                pend = [None, None, None]   # quads qd-1, qd-2, qd-3
                for kc in range(NKC):
                    kslc = slice(kc * 128, (kc + 1) * 128)
                    nbb = nbb_ready.pop((b, kc))
                    # prepare the NEXT kc's bias one full kc ahead so the
                    # pre-add never sits on the critical path (including
                    # across the batch boundary)
                    nb2, nkc2 = (b, kc + 1) if kc + 1 < NKC else (b + 1, 0)
                    if nb2 < BLOC:
                        prep_bias(nb2, nkc2)
                    for qd in range(2):       # head quad: 4*qd .. 4*qd+3
                        heads = [4 * qd + i for i in range(4)]
                        qi = kc * 2 + qd
                        if pending_post is not None and qi < 3:
                            pending_post[qi]()
                            if qi == 2:
                                pending_post = None
                        # 4-way row-tiled QK^T into two 2-bank tiles
                        qk2a = psp.tile([128, 2, Q], F32, tag="mm")
                        qk2b = psp.tile([128, 2, Q], F32, tag="mm")
                        qkt = (qk2a, qk2b)
                        for i, h in enumerate(heads):
                            j = h % 4
                            jslc = slice(32 * j, 32 * j + 32)
                            nc.tensor.matmul(
                                qkt[i // 2][:, i % 2],
                                khT[jslc, h // 4, kslc],
                                qhT[jslc, h // 4],
                                start=True, stop=False,
                                tile_position=(32 * j, 0))
                        # identity-add of biases into psum
                        for i, h in enumerate(heads):
                            nc.tensor.matmul(
                                qkt[i // 2][:, i % 2], ident, nbb[:, h],
                                start=False, stop=True)
                        # AV of quad-2 and sums of quad-3 fill the PE while
                        # this quad's exp runs (extra lag so the first AV of
                        # a batch never blocks the PE FIFO on the previous
                        # batch's post chain); at mid-batch the lag collapses
                        # to 1/2 so the end-of-batch drain stays short
                        if qi < 4:
                            if pend[1] is not None:
                                emit_av(pend[1])
                            if pend[2] is not None:
                                emit_sums(pend[2])
                        elif qi == 4:
                            emit_av(pend[1])
                            emit_av(pend[0])
                            emit_sums(pend[2])
                            emit_sums(pend[1])
                        else:
                            emit_av(pend[0])
                            emit_sums(pend[1])
                        # exp (+ per-key row bias) -> bf16
                        ptq = ptp.tile([128, 4, Q], BF16, tag="pt", bufs=6)
                        nc.scalar.activation(ptq[:, 0:2], qk2a, AF.Exp,
                                             bias=br_sb[:, kc:kc + 1], scale=1.0)
                        nc.scalar.activation(ptq[:, 2:4], qk2b, AF.Exp,
                                             bias=br_sb[:, kc:kc + 1], scale=1.0)
                        pts = [ptq[:, i] for i in range(4)]
                        pend = [(heads, pts, kc), pend[0], pend[1]]
                # drain (lag is 1/2 by now): av of the last quad, sums of
                # the last two
                emit_av(pend[0])
                emit_sums(pend[1])
                emit_sums(pend[0])
                pending_post = make_post(b, avt, smt, gate)
            for piece in pending_post:
                piece()

    nc.compile()
    return nc


def make_in_maps(q_data, m_data, bias, nonbatched_bias, batched_bias,
                 query_w, key_w, value_w, gating_w, gating_b, output_w, output_b):
    """Host-side layout prep (transpose/reshape only) + sharding over 8 cores.

    All tensors are packed so device DMA lines (innermost per-partition
    runs) are contiguous and >= 2KB.
    """
    import ml_dtypes
    f = np.float32
    bf = ml_dtypes.bfloat16
    # [B,Q,A] -> [B, p(128), ka(2), Q]
    qT = np.ascontiguousarray(
        np.asarray(q_data, f).transpose(0, 2, 1).reshape(B, 2, 128, Q)
        .transpose(0, 2, 1, 3).astype(bf))
    mT = np.ascontiguousarray(
        np.asarray(m_data, f).transpose(0, 2, 1).reshape(B, 2, 128, K)
        .transpose(0, 2, 1, 3).astype(bf))
    # [B,1,1,K] -> [B, p(128), kc(4)]
    br = np.ascontiguousarray(
        np.asarray(bias, f).reshape(B, NKC, 128).transpose(0, 2, 1))
    # [B,H,Q,K] -> [B, kc, p, H, Q]
    bbT = np.ascontiguousarray(
        np.asarray(batched_bias, f).transpose(0, 3, 1, 2)
        .reshape(B, NKC, 128, H, Q).astype(bf))
    # [H,Q,K] -> [p, kc, H, Q]
    nbT = np.ascontiguousarray(
        np.asarray(nonbatched_bias, f).transpose(2, 0, 1)
        .reshape(NKC, 128, H, Q).transpose(1, 0, 2, 3).astype(bf))
    # [A,H,C] -> [p, ka, hc]
    def wpack(w):
        return np.ascontiguousarray(
            np.asarray(w, f).reshape(2, 128, H * C).transpose(1, 0, 2).astype(bf))
    qw, kw, vw, gw = wpack(query_w), wpack(key_w), wpack(value_w), wpack(gating_w)
    # [H,C,O] -> [p, kh, o]
    ow = np.ascontiguousarray(
        np.asarray(output_w, f).reshape(2, 128, O).transpose(1, 0, 2).astype(bf))
    gb = np.ascontiguousarray(np.asarray(gating_b, f).reshape(2, 128).T)
    ob = np.ascontiguousarray(np.asarray(output_b, f).reshape(2, 128).T)
    in_maps = []
    for c in range(CORES):
        s = slice(c * BLOC, (c + 1) * BLOC)
        in_maps.append({
            "qT": qT[s], "mT": mT[s], "biasr": br[s], "bbT": bbT[s], "nbT": nbT,
            "qw": qw, "kw": kw, "vw": vw, "gw": gw, "gb": gb, "ow": ow, "ob": ob,
        })
    return in_maps


_NC_CACHE = {}


def get_nc():
    if "nc" not in _NC_CACHE:
        _NC_CACHE["nc"] = build_nc()
    return _NC_CACHE["nc"]


def unpack_out(res_out):
    """[BLOC, p, mo, q] -> [BLOC, Q, O]"""
    return np.ascontiguousarray(
        np.asarray(res_out, dtype=np.float32).reshape(BLOC, 128, 2, Q)
        .transpose(0, 3, 2, 1).reshape(BLOC, Q, O))


def kernel(**inputs):
    in_maps = make_in_maps(**inputs)
    nc = get_nc()
    res = run_bass_kernel_spmd(nc, in_maps, core_ids=list(range(CORES)))
    outs = [unpack_out(res.results[c]["out"]) for c in range(CORES)]
    return np.ascontiguousarray(np.concatenate(outs, axis=0))


# revision 54
# speedup vs baseline: 1.1346x; 1.1346x over previous
"""Gated multi-head attention (AlphaFold-style) on 8 TRN2 NeuronCores.

Sharding: data-parallel over batch B=32 -> 4 batches per core; zero collectives.

v2 over the v1 baseline (145us):
  - All DMA layouts packed host-side so every per-partition line is
    contiguous (2KB-32KB): one DMA per (batch, kc) for batched bias
    (8KB lines), one DMA for the whole nonbatched bias (32KB lines).
  - Bias pre-add (nbb = nbT + bbT) split by head quad: heads 0-3 on DVE
    (bf16 2x mode), heads 4-7 on GPSIMD (otherwise idle engine).
  - Softmax denominators: the ones-matmuls now target the column bands
    the AV matmuls are NOT using (band j^2), so all 4 matmuls of an
    emit_av group run concurrently in the PE array. Sums land with
    32-row bands swapped (rows 64-127 hold heads' j=0,1 sums); the
    post chain compensates with 64-partition-offset DVE operands.
  - PSUM->SBUF evacuation copies (qhT/khT/vb) moved to ACT (idle during
    the projection phase); sigmoid/copies merged into wider single ops.

  qhT[hc, q]  = (query_w*scale)[a,hc]^T @ q_dataT[a,q]           (PE)
  khT[hc, k]  =  key_w^T @ m_dataT                               (PE)
  vb[k, hc]   =  (m_dataT^T-chunks @ value_w) -> bf16            (PE + ACT)
  gateT[hc,q] =  sigmoid(gating_w^T @ q_dataT + gating_b)        (PE + ACT)
  logitsT(h)[k,q] = khT_h^T-slices @ qhT_h  (row-tiled)          (PE)
  psum += Id @ (nbT + bbT)               (PE id-add; DVE/GPS pre-add)
  PT(h)[k,q]  = exp(psum + bias_row[k])  -> bf16                 (ACT)
  avT, sums   = col-tiled matmuls over k, all 4 bands concurrent (PE)
  wag         = avT * gateT * approx(1/sums)                     (DVE)
  outT[o, q]  = output_w^T-chunks @ wag + output_b               (PE + DVE)
"""

import numpy as np

import concourse.bass as bass
import concourse.mybir as mybir
from concourse import bacc
from concourse.tile import TileContext
from concourse.masks import make_identity
from concourse.bass_utils import run_bass_kernel_spmd

B, Q, K, A, H, C, O = 32, 512, 512, 256, 8, 32, 256
CORES = 8
BLOC = B // CORES          # batches per core
NKC = K // 128             # k chunks
F32 = mybir.dt.float32
BF16 = mybir.dt.bfloat16
KEY_SCALE = float(C) ** -0.5
AF = mybir.ActivationFunctionType
ALU = mybir.AluOpType


def build_nc():
    nc = bacc.Bacc(None, target_bir_lowering=False)

    # --- DRAM parameters (per-core shards; packed layouts, see make_in_maps) ---
    p_qT = nc.declare_dram_parameter("qT", [BLOC, 128, 2, Q], BF16, isOutput=False)
    p_mT = nc.declare_dram_parameter("mT", [BLOC, 128, 2, K], BF16, isOutput=False)
    p_br = nc.declare_dram_parameter("biasr", [BLOC, 128, NKC], F32, isOutput=False)
    p_bbT = nc.declare_dram_parameter("bbT", [BLOC, NKC, 128, H, Q], BF16,
                                      isOutput=False)
    p_nbT = nc.declare_dram_parameter("nbT", [128, NKC, H, Q], BF16, isOutput=False)
    p_qw = nc.declare_dram_parameter("qw", [128, 2, 256], BF16, isOutput=False)
    p_kw = nc.declare_dram_parameter("kw", [128, 2, 256], BF16, isOutput=False)
    p_vw = nc.declare_dram_parameter("vw", [128, 2, 256], BF16, isOutput=False)
    p_gw = nc.declare_dram_parameter("gw", [128, 2, 256], BF16, isOutput=False)
    p_gb = nc.declare_dram_parameter("gb", [128, 2], F32, isOutput=False)
    p_ow = nc.declare_dram_parameter("ow", [128, 2, 256], BF16, isOutput=False)
    p_ob = nc.declare_dram_parameter("ob", [128, 2], F32, isOutput=False)
    p_out = nc.declare_dram_parameter("out", [BLOC, 128, 2, Q], BF16, isOutput=True)

    with TileContext(nc) as tc:
        with (
            tc.tile_pool(name="const", bufs=1) as const,
            tc.tile_pool(name="nbres", bufs=1) as nbres,
            tc.tile_pool(name="data", bufs=4) as data,
            tc.tile_pool(name="proj", bufs=4) as proj,
            tc.tile_pool(name="bbt", bufs=4) as bbtp,
            tc.tile_pool(name="nbb", bufs=4) as nbbp,
            tc.tile_pool(name="pt", bufs=8) as ptp,
            tc.tile_pool(name="post", bufs=1) as post,
            tc.tile_pool(name="ps", bufs=3, space="PSUM") as psp,
            tc.tile_pool(name="avps", bufs=1, space="PSUM") as avps,
            tc.tile_pool(name="sumps", bufs=1, space="PSUM") as sumps,
        ):
            # ---------- one-time constants ----------
            nbt16 = nbres.tile([128, NKC, H, Q], BF16)

            ident = const.tile([128, 128], BF16)
            make_identity(nc, ident)
            ones = const.tile([128, 32], BF16)
            nc.vector.memset(ones, 1.0)

            # weights: packed [128, ka, hc]; cast to bf16 on device;
            # key_scale folded into qw here.
            qw_sb = const.tile([128, 2, 256], BF16)
            kw_sb = const.tile([128, 2, 256], BF16)
            vw_sb = const.tile([128, 2, 256], BF16)
            gw_sb = const.tile([128, 2, 256], BF16)
            ow_sb = const.tile([128, 2, 256], BF16)
            # trigger order follows first-use: qw/kw/gw gate the very
            # first projection matmuls, vw/gb follow within ~3us; ow/ob are
            # not needed until the first post (~60us) and load after the
            # per-batch inputs below.
            wstage = data.tile([128, 2, 256], BF16, tag="stage", bufs=1)
            nc.sync.dma_start(out=wstage, in_=p_qw[:])
            nc.sync.dma_start(out=kw_sb, in_=p_kw[:])
            nc.sync.dma_start(out=gw_sb, in_=p_gw[:])
            nc.vector.tensor_scalar_mul(out=qw_sb, in0=wstage, scalar1=KEY_SCALE)
            nc.sync.dma_start(out=vw_sb, in_=p_vw[:])
            gb_sb = const.tile([128, 2], F32)
            nc.sync.dma_start(out=gb_sb, in_=p_gb[:])
            ob_sb = const.tile([128, 2], F32)
            # negated gating bias: the gate is computed through the Exp
            # table (sigmoid(x) = 1/(1+exp(-x))) so the attention phase
            # never needs an ACT table switch
            ngb_sb = const.tile([128, 2], F32)
            nc.vector.tensor_scalar_mul(out=ngb_sb, in0=gb_sb, scalar1=-1.0)

            # ---------- per-batch pipeline ----------
            # The post chain is split into pieces emitted across the first
            # quads of the NEXT batch so its DVE work never forms a solid
            # block in front of that batch's bias pre-adds.
            def make_post(b, avt, smt, gate):
                recb = [None]
                wag = [None]

                def piece1():
                    recb[0] = post.tile([128, 2, Q], F32, tag="recb", name="recb")
                    grec = post.tile([128, 2, Q], F32, tag="grec")
                    for t in range(2):
                        nc.vector.reciprocal_approx_fast(
                            out=recb[0][:, t], in_=smt[t])
                    for t in range(2):
                        nc.vector.tensor_mul(
                            out=grec[:, t], in0=gate[:, t], in1=recb[0][:, t])
                    recb[0] = grec

                def piece2():
                    grec = recb[0]
                    wag[0] = post.tile([128, 2, Q], BF16, tag="wag", name="wag")
                    for t in range(2):
                        nc.vector.tensor_mul(
                            out=wag[0][:, t], in0=avt[t], in1=grec[:, t])

                def piece3():
                    outT = post.tile([128, 2, Q], BF16, tag="outT", bufs=2)
                    po2 = psp.tile([128, 2, Q], F32, tag="mm")
                    for mo in range(2):
                        oslc = slice(mo * 128, (mo + 1) * 128)
                        for kh in range(2):
                            nc.tensor.matmul(
                                po2[:, mo], ow_sb[:, kh, oslc], wag[0][:, kh],
                                start=(kh == 0), stop=(kh == 1))
                    for mo in range(2):
                        nc.scalar.add(out=outT[:, mo], in_=po2[:, mo],
                                      add=ob_sb[:, mo:mo + 1])
                    nc.gpsimd.dma_start(out=p_out[b], in_=outT)

                return [piece1, piece2, piece3]

            # ---------- hoisted input DMA loads ----------
            br_l, qT_l, mT_l = [], [], []
            for b in range(BLOC):
                qT_sb = data.tile([128, 2, Q], BF16, tag="qT", bufs=4)
                nc.sync.dma_start(out=qT_sb, in_=p_qT[b])
                mT_sb = data.tile([128, 2, K], BF16, tag="mT", bufs=2)
                nc.sync.dma_start(out=mT_sb, in_=p_mT[b])
                br_sb = data.tile([128, NKC], F32, tag="br")
                nc.sync.dma_start(out=br_sb, in_=p_br[b])
                qT_l.append(qT_sb)
                mT_l.append(mT_sb)
                br_l.append(br_sb)
                if b == 0:
                    # batch 0's attention starts right after proj(0); its
                    # first bias pre-adds need the kc0 chunk early
                    nc.sync.dma_start(out=nbt16[:, 0, 0:4], in_=p_nbT[:, 0, 0:4])
                    nc.sync.dma_start(out=nbt16[:, 0, 4:8], in_=p_nbT[:, 0, 4:8])
            nc.sync.dma_start(out=ow_sb, in_=p_ow[:])
            nc.sync.dma_start(out=ob_sb, in_=p_ob[:])
            for kc in range(1, NKC):
                nc.sync.dma_start(out=nbt16[:, kc, 0:4], in_=p_nbT[:, kc, 0:4])
                nc.sync.dma_start(out=nbt16[:, kc, 4:8], in_=p_nbT[:, kc, 4:8])

            # ---------- per-batch projections ----------
            # Emitted at the tail of the previous batch's attention: the PE
            # fills the batch-boundary bubble with proj matmuls (HAM stays
            # warm), ACT takes the PSUM evacuations there (it idles at the
            # boundary), and the gate goes through Exp so no table switch.
            def make_proj(b, use_sigmoid=True):
                qT_sb, mT_sb = qT_l[b], mT_l[b]
                qhT = proj.tile([128, 2, Q], BF16, tag="qhT", bufs=4, name="qhT")
                khT = proj.tile([128, 2, K], BF16, tag="khT", bufs=4, name="khT")
                gate = proj.tile([128, 2, Q], F32, tag="gate", bufs=4, name="gate")
                pqh = psp.tile([128, 2, Q], F32, tag="mm", name="pqh")
                pkh = psp.tile([128, 2, Q], F32, tag="mm", name="pkh")
                pgh = psp.tile([128, 2, Q], F32, tag="mm", name="pgh")
                for m in range(2):
                    mslc = slice(m * 128, (m + 1) * 128)
                    for ka in range(2):
                        st, sp = ka == 0, ka == 1
                        nc.tensor.matmul(
                            pqh[:, m], qw_sb[:, ka, mslc], qT_sb[:, ka],
                            start=st, stop=sp)
                        nc.tensor.matmul(
                            pkh[:, m], kw_sb[:, ka, mslc], mT_sb[:, ka],
                            start=st, stop=sp)
                        nc.tensor.matmul(
                            pgh[:, m], gw_sb[:, ka, mslc], qT_sb[:, ka],
                            start=st, stop=sp)
                nc.vector.tensor_copy(out=qhT, in_=pqh)
                nc.vector.tensor_copy(out=khT, in_=pkh)
                if use_sigmoid:
                    for m in range(2):
                        nc.scalar.activation(gate[:, m], pgh[:, m], AF.Sigmoid,
                                             bias=gb_sb[:, m:m + 1], scale=1.0)
                else:
                    # gate via the resident Exp table: 1/(1+exp(-(x+gb)))
                    for m in range(2):
                        nc.scalar.activation(gate[:, m], pgh[:, m], AF.Exp,
                                             bias=ngb_sb[:, m:m + 1], scale=-1.0)
                    nc.vector.tensor_scalar_add(out=gate, in0=gate, scalar1=1.0)
                    nc.vector.reciprocal_approx_fast(out=gate, in_=gate)

                vb = proj.tile([128, NKC, 256], BF16, tag="vb", bufs=4, name="vb")
                for kch in range(2):
                    pv2 = psp.tile([128, 2, Q], F32, tag="mm", name="pv2")
                    for kci in range(2):
                        kc = 2 * kch + kci
                        kslc = slice(kc * 128, (kc + 1) * 128)
                        pv = pv2[:, kci, 0:256]
                        for ka in range(2):
                            nc.tensor.matmul(
                                pv, mT_sb[:, ka, kslc], vw_sb[:, ka],
                                start=(ka == 0), stop=(ka == 1))
                    nc.vector.tensor_copy(out=vb[:, 2 * kch:2 * kch + 2],
                                       in_=pv2[:, :, 0:256])
                return qhT, khT, gate, vb

            nbb_ready = {}

            def prep_bias(b2, kc2):
                bbt = bbtp.tile([128, H, Q], BF16, tag="bbt", name="bbt")
                nc.sync.dma_start(out=bbt, in_=p_bbT[b2, kc2])
                nbb = nbbp.tile([128, H, Q], BF16, tag="nbb", name="nbb")
                nc.vector.tensor_add(
                    out=nbb[:, 0:4], in0=nbt16[:, kc2, 0:4], in1=bbt[:, 0:4])
                nc.vector.tensor_add(
                    out=nbb[:, 4:8], in0=nbt16[:, kc2, 4:8], in1=bbt[:, 4:8])
                nbb_ready[(b2, kc2)] = nbb

            projd_l = [make_proj(0), make_proj(1), None, None]
            prep_bias(0, 0)
            pending_post = None
            for b in range(BLOC):
                if b == 1:
                    # emitted here, these run inside attn(b0)'s PE slack and
                    # their sigmoids land in the b0->b1 boundary bubble
                    projd_l[2] = make_proj(2)
                    projd_l[3] = make_proj(3)
                qhT, khT, gate, vb = projd_l[b]
                br_sb = br_l[b]

                # --- attention core ---
                av0 = avps.tile([128, Q], F32, tag="av")     # heads 0-3
                av1 = avps.tile([128, Q], F32, tag="av")     # heads 4-7
                sm0 = sumps.tile([128, Q], F32, tag="sm")    # band-swapped
                sm1 = sumps.tile([128, Q], F32, tag="sm")
                avt = (av0, av1)
                smt = (sm0, sm1)

                def emit_av(g):
                    # 4 heads in 4 distinct column bands -> concurrent.
                    g_heads, g_pts, g_kc = g
                    for i2, h2 in enumerate(g_heads):
                        j2 = h2 % 4
                        nc.tensor.matmul(
                            avt[h2 // 4][32 * j2:32 * j2 + 32],
                            vb[:, g_kc, 32 * h2:32 * h2 + 32],
                            g_pts[i2],
                            start=(g_kc == 0), stop=(g_kc == NKC - 1),
                            tile_position=(0, 32 * j2), skip_group_check=True)

                def emit_sums(g):
                    g_heads, g_pts, g_kc = g
                    for i2, h2 in enumerate(g_heads):
                        j2 = h2 % 4
                        nc.tensor.matmul(
                            smt[h2 // 4][32 * j2:32 * j2 + 32],
                            ones, g_pts[i2],
                            start=(g_kc == 0), stop=(g_kc == NKC - 1),
                            tile_position=(0, 32 * j2), skip_group_check=True)

                pend = [None, None, None]   # quads qd-1, qd-2, qd-3
                for kc in range(NKC):
                    kslc = slice(kc * 128, (kc + 1) * 128)
                    nbb = nbb_ready.pop((b, kc))
                    # prepare the NEXT kc's bias one full kc ahead so the
                    # pre-add never sits on the critical path (including
                    # across the batch boundary)
                    nb2, nkc2 = (b, kc + 1) if kc + 1 < NKC else (b + 1, 0)
                    if nb2 < BLOC:
                        prep_bias(nb2, nkc2)
                    for qd in range(2):       # head quad: 4*qd .. 4*qd+3
                        heads = [4 * qd + i for i in range(4)]
                        qi = kc * 2 + qd
                        if pending_post is not None and qi < 3:
                            pending_post[qi]()
                            if qi == 2:
                                pending_post = None
                        # 4-way row-tiled QK^T into two 2-bank tiles
                        qk2a = psp.tile([128, 2, Q], F32, tag="mm")
                        qk2b = psp.tile([128, 2, Q], F32, tag="mm")
                        qkt = (qk2a, qk2b)
                        for i, h in enumerate(heads):
                            j = h % 4
                            jslc = slice(32 * j, 32 * j + 32)
                            nc.tensor.matmul(
                                qkt[i // 2][:, i % 2],
                                khT[jslc, h // 4, kslc],
                                qhT[jslc, h // 4],
                                start=True, stop=False,
                                tile_position=(32 * j, 0))
                        # identity-add of biases into psum
                        for i, h in enumerate(heads):
                            nc.tensor.matmul(
                                qkt[i // 2][:, i % 2], ident, nbb[:, h],
                                start=False, stop=True)
                        # AV of quad-2 and sums of quad-3 fill the PE while
                        # this quad's exp runs (extra lag so the first AV of
                        # a batch never blocks the PE FIFO on the previous
                        # batch's post chain); at mid-batch the lag collapses
                        # to 1/2 so the end-of-batch drain stays short
                        if qi < 4:
                            if pend[1] is not None:
                                emit_av(pend[1])
                            if pend[2] is not None:
                                emit_sums(pend[2])
                        elif qi == 4:
                            emit_av(pend[1])
                            emit_av(pend[0])
                            emit_sums(pend[2])
                            emit_sums(pend[1])
                        else:
                            emit_av(pend[0])
                            emit_sums(pend[1])
                        # exp (+ per-key row bias) -> bf16
                        ptq = ptp.tile([128, 4, Q], BF16, tag="pt", bufs=6)
                        nc.scalar.activation(ptq[:, 0:2], qk2a, AF.Exp,
                                             bias=br_sb[:, kc:kc + 1], scale=1.0)
                        nc.scalar.activation(ptq[:, 2:4], qk2b, AF.Exp,
                                             bias=br_sb[:, kc:kc + 1], scale=1.0)
                        pts = [ptq[:, i] for i in range(4)]
                        pend = [(heads, pts, kc), pend[0], pend[1]]
                # drain (lag is 1/2 by now): av of the last quad, sums of
                # the last two
                emit_av(pend[0])
                emit_sums(pend[1])
                emit_sums(pend[0])
                pending_post = make_post(b, avt, smt, gate)
            for piece in pending_post:
                piece()

    nc.compile()
    return nc


def make_in_maps(q_data, m_data, bias, nonbatched_bias, batched_bias,
                 query_w, key_w, value_w, gating_w, gating_b, output_w, output_b):
    """Host-side layout prep (transpose/reshape only) + sharding over 8 cores.

    All tensors are packed so device DMA lines (innermost per-partition
    runs) are contiguous and >= 2KB.
    """
    import ml_dtypes
    f = np.float32
    bf = ml_dtypes.bfloat16
    # [B,Q,A] -> [B, p(128), ka(2), Q]
    qT = np.ascontiguousarray(
        np.asarray(q_data, f).transpose(0, 2, 1).reshape(B, 2, 128, Q)
        .transpose(0, 2, 1, 3).astype(bf))
    mT = np.ascontiguousarray(
        np.asarray(m_data, f).transpose(0, 2, 1).reshape(B, 2, 128, K)
        .transpose(0, 2, 1, 3).astype(bf))
    # [B,1,1,K] -> [B, p(128), kc(4)]
    br = np.ascontiguousarray(
        np.asarray(bias, f).reshape(B, NKC, 128).transpose(0, 2, 1))
    # [B,H,Q,K] -> [B, kc, p, H, Q]
    bbT = np.ascontiguousarray(
        np.asarray(batched_bias, f).transpose(0, 3, 1, 2)
        .reshape(B, NKC, 128, H, Q).astype(bf))
    # [H,Q,K] -> [p, kc, H, Q]
    nbT = np.ascontiguousarray(
        np.asarray(nonbatched_bias, f).transpose(2, 0, 1)
        .reshape(NKC, 128, H, Q).transpose(1, 0, 2, 3).astype(bf))
    # [A,H,C] -> [p, ka, hc]
    def wpack(w):
        return np.ascontiguousarray(
            np.asarray(w, f).reshape(2, 128, H * C).transpose(1, 0, 2).astype(bf))
    qw, kw, vw, gw = wpack(query_w), wpack(key_w), wpack(value_w), wpack(gating_w)
    # [H,C,O] -> [p, kh, o]
    ow = np.ascontiguousarray(
        np.asarray(output_w, f).reshape(2, 128, O).transpose(1, 0, 2).astype(bf))
    gb = np.ascontiguousarray(np.asarray(gating_b, f).reshape(2, 128).T)
    ob = np.ascontiguousarray(np.asarray(output_b, f).reshape(2, 128).T)
    in_maps = []
    for c in range(CORES):
        s = slice(c * BLOC, (c + 1) * BLOC)
        in_maps.append({
            "qT": qT[s], "mT": mT[s], "biasr": br[s], "bbT": bbT[s], "nbT": nbT,
            "qw": qw, "kw": kw, "vw": vw, "gw": gw, "gb": gb, "ow": ow, "ob": ob,
        })
    return in_maps


_NC_CACHE = {}


def get_nc():
    if "nc" not in _NC_CACHE:
        _NC_CACHE["nc"] = build_nc()
    return _NC_CACHE["nc"]


def unpack_out(res_out):
    """[BLOC, p, mo, q] -> [BLOC, Q, O]"""
    return np.ascontiguousarray(
        np.asarray(res_out, dtype=np.float32).reshape(BLOC, 128, 2, Q)
        .transpose(0, 3, 2, 1).reshape(BLOC, Q, O))


def kernel(**inputs):
    in_maps = make_in_maps(**inputs)
    nc = get_nc()
    res = run_bass_kernel_spmd(nc, in_maps, core_ids=list(range(CORES)))
    outs = [unpack_out(res.results[c]["out"]) for c in range(CORES)]
    return np.ascontiguousarray(np.concatenate(outs, axis=0))
